# revision 1
# baseline (speedup 1.0000x reference)
"""GIN (3-layer) over a 100K-node/1.6M-edge graph on 8 TRN2 NeuronCores.

Sharding: node g -> core g % 8 (interleaved so every core owns exactly
12500 real nodes + 44 pad slots). Each core owns its nodes' aggregation:
edges are routed to the core owning the *destination* (col for GIN
message passing, row for the edge_attr scatter-mean). Per-core node
slots are in-degree-sorted so the per-tile gather depth is uniform.

Per GIN layer: h is AllGathered into a replicated DRAM table; each core
issues one 128-row indirect DMA per slot column (the measured-optimal
gather primitive on this hardware), reduces each 128-dest tile on DVE,
then runs the MLP feature-major on the TensorEngine (transpose, two
matmuls, ACT bias+ReLU), computes BN stats with ACT accum_out, and
AllReduces stats. Final global mean pool is a free-dim reduce +
AllReduce + replicated classifier.
"""

import numpy as np
import ml_dtypes

import concourse.bass as bass
import concourse.bacc as bacc
import concourse.tile as tile
import concourse.mybir as mybir
from concourse.bass import IndirectOffsetOnAxis
from concourse.bass_utils import run_bass_kernel_spmd
from concourse.masks import make_identity

F32 = mybir.dt.float32
BF16 = mybir.dt.bfloat16
I32 = mybir.dt.int32

N_CORES = 8
FN, FE, H, C = 32, 16, 64, 2
BN_EPS = 1e-5


class Cfg:
    def __init__(self, n_real, local):
        assert local % 128 == 0
        self.n_real = n_real              # real nodes in the graph
        self.local = local                # slots per core (multiple of 128)
        self.real_local = n_real // N_CORES  # real nodes per core
        self.nt = local // 128            # 128-dest tiles per core
        self.gn = N_CORES * local         # rows in the gathered h table


FULL = Cfg(100_000, 12_544)


# --------------------------------------------------------------------------
# Host preprocessing: slot layout, gather indices, edge_attr relay
# --------------------------------------------------------------------------

def host_prep(inputs, cfg):
    E = inputs["edge_index"].shape[1]
    row = np.asarray(inputs["edge_index"][0])
    col = np.asarray(inputs["edge_index"][1])
    L, NT = cfg.local, cfg.nt

    # Global in-degree rank r -> core r % 8, slot r // 8: every core's
    # tile t covers the same global rank range, so the cross-core tile
    # max (which pads the gather depth) collapses to the in-tile spread.
    degg = np.bincount(col, minlength=cfg.n_real)
    grank = np.argsort(-degg, kind="stable")
    rank_of = np.empty(cfg.n_real, np.int64)
    rank_of[grank] = np.arange(cfg.n_real)
    core_of = (rank_of % N_CORES).astype(np.int32)
    slot_of_global = (rank_of // N_CORES).astype(np.int32)
    degs = np.zeros((N_CORES, L), np.int64)
    degs[core_of, slot_of_global] = degg

    # Per-tile gather depth, shared across cores (SPMD graph is shared).
    D_t = degs.reshape(N_CORES, NT, 128).max(axis=(0, 2)).astype(np.int64)
    D_t = np.maximum(D_t, 0)
    S = int(D_t.sum())

    # table row of source node g' in the AllGathered h table
    tab_row = (core_of.astype(np.int64) * L
               + slot_of_global).astype(np.int32)

    # gather index array per core: [128, S]; column block t has D_t columns
    pad_row = cfg.real_local          # core 0's first virtual slot: always 0
    gidx = np.full((N_CORES, 128, max(S, 1)), pad_row, np.int32)
    Dmax = int(D_t.max()) if S else 0
    col_off = np.zeros(NT, np.int64)
    np.cumsum(D_t[:-1], out=col_off[1:])
    for c in range(N_CORES):
        m = core_of[col] == c
        e = np.where(m)[0]
        slot_e = slot_of_global[col[e]]
        o = np.argsort(slot_e, kind="stable")
        e, slot_e = e[o], slot_e[o]
        # j = rank of edge within its slot
        start = np.zeros(L + 1, np.int64)
        np.cumsum(np.bincount(slot_e, minlength=L), out=start[1:])
        j = np.arange(len(e)) - start[slot_e]
        A = np.full((L, Dmax), pad_row, np.int32)
        A[slot_e, j] = tab_row[row[e]]
        for t in range(NT):
            d = int(D_t[t])
            if d:
                gidx[c, :, col_off[t]:col_off[t] + d] = A[t * 128:(t + 1) * 128, :d]

    # edge_attr scatter-mean relay (dest = row): per-slot row-degree layout
    rdegs = []
    slotr = []
    for c in range(N_CORES):
        m = core_of[row] == c
        e = np.where(m)[0]
        slot_e = slot_of_global[row[e]]
        rd = np.bincount(slot_e, minlength=L)
        rdegs.append(rd)
        slotr.append((e, slot_e))
    rdegs = np.stack(rdegs)
    Dp_t = rdegs.reshape(N_CORES, NT, 128).max(axis=(0, 2)).astype(np.int64)
    SEA = int(Dp_t.sum())
    ea_off = np.zeros(NT, np.int64)
    np.cumsum(Dp_t[:-1], out=ea_off[1:])
    Dpmax = int(Dp_t.max()) if SEA else 0

    ea = np.asarray(inputs["edge_attr"]).astype(ml_dtypes.bfloat16)
    ea_st = np.zeros((N_CORES, 128, max(SEA, 1) * FE), ml_dtypes.bfloat16)
    icnt = np.zeros((N_CORES, 128, NT), np.float32)
    for c in range(N_CORES):
        e, slot_e = slotr[c]
        o = np.argsort(slot_e, kind="stable")
        e, slot_e = e[o], slot_e[o]
        start = np.zeros(L + 1, np.int64)
        np.cumsum(np.bincount(slot_e, minlength=L), out=start[1:])
        j = np.arange(len(e)) - start[slot_e]
        A = np.zeros((L, Dpmax, FE), ml_dtypes.bfloat16)
        A[slot_e, j, :] = ea[e]
        for t in range(NT):
            d = int(Dp_t[t])
            if d:
                blk = A[t * 128:(t + 1) * 128, :d, :].reshape(128, d * FE)
                ea_st[c, :, ea_off[t] * FE:(ea_off[t] + d) * FE] = blk
        icnt[c, :, :] = (1.0 / np.maximum(
            rdegs[c].reshape(NT, 128), 1.0)).T.astype(np.float32)

    # node features in slot order
    x = np.asarray(inputs["x"], np.float32)
    xp = np.zeros((N_CORES, L, FN), np.float32)
    xp[core_of, slot_of_global] = x

    meta = dict(D_t=D_t, S=S, Dp_t=Dp_t, SEA=SEA,
                col_off=col_off, ea_off=ea_off)
    return gidx, ea_st, icnt, xp, meta


# --------------------------------------------------------------------------
# Device program
# --------------------------------------------------------------------------

WNAMES = []
for _l in range(3):
    WNAMES += [f"w1_{_l}", f"b1_{_l}", f"w2_{_l}", f"b2_{_l}", f"g_{_l}", f"be_{_l}"]
WNAMES += ["cw1", "cb1", "cw2", "cb2"]


def build_program(cfg, meta, reps=1):
    L, NT, GN = cfg.local, cfg.nt, cfg.gn
    D_t, S = meta["D_t"], meta["S"]
    Dp_t, SEA = meta["Dp_t"], meta["SEA"]
    col_off, ea_off = meta["col_off"], meta["ea_off"]
    inv_n = 1.0 / cfg.n_real

    nc = bacc.Bacc("TRN2", target_bir_lowering=False, debug=False,
                   num_devices=N_CORES)

    gidx_in = nc.dram_tensor("gidx", [128, max(S, 1)], I32, kind="ExternalInput")
    ea_in = nc.dram_tensor("ea_st", [128, max(SEA, 1) * FE], BF16,
                           kind="ExternalInput")
    icnt_in = nc.dram_tensor("icnt", [128, NT], F32, kind="ExternalInput")
    xp_in = nc.dram_tensor("xp", [L, FN], F32, kind="ExternalInput")
    wts = {}
    wshapes = {}
    for l in range(3):
        kin = FN + FE if l == 0 else H
        wshapes[f"w1_{l}"] = [kin, H]
        wshapes[f"b1_{l}"] = [H]
        wshapes[f"w2_{l}"] = [H, H]
        wshapes[f"b2_{l}"] = [H]
        wshapes[f"g_{l}"] = [H]
        wshapes[f"be_{l}"] = [H]
    wshapes.update(cw1=[H, H // 2], cb1=[H // 2], cw2=[H // 2, C], cb2=[C])
    for n in WNAMES:
        wts[n] = nc.dram_tensor(n, wshapes[n], F32, kind="ExternalInput")
    out_ext = nc.dram_tensor("out", [1, C], F32, kind="ExternalOutput")

    K0 = FN + FE
    rep = [list(range(N_CORES))]

    with tile.TileContext(nc) as tc:
        with (
            tc.tile_pool(name="dram", bufs=1, space="DRAM") as dram,
            tc.tile_pool(name="persist", bufs=1) as ps,
            tc.tile_pool(name="gpool", bufs=8) as gp,
            tc.tile_pool(name="ea", bufs=3) as eap,
            tc.tile_pool(name="fm", bufs=3) as fm,
            tc.tile_pool(name="small", bufs=2) as sm,
            tc.tile_pool(name="psum", bufs=2, space="PSUM") as pp,
            tc.tile_pool(name="psum2", bufs=2, space="PSUM") as pp2,
        ):

            # ---- persistent SBUF
            idx_sb = ps.tile([128, max(S, 1)], I32)
            nc.sync.dma_start(out=idx_sb[:], in_=gidx_in[:])
            h_own = ps.tile([128, NT, H], BF16)      # node-major h (l0: 48)
            acc = ps.tile([128, NT, H], F32)         # agg / z node-major
            z2T = ps.tile([H, L], F32)               # feature-major MLP out
            icnt_sb = ps.tile([128, NT], F32)
            nc.sync.dma_start(out=icnt_sb[:], in_=icnt_in[:])
            ident = ps.tile([128, 128], F32)
            make_identity(nc, ident[:])
            wt = {}
            for n in WNAMES:
                shp = wshapes[n]
                if len(shp) == 1:
                    wt[n] = ps.tile([shp[0], 1], F32, name=f"wt_{n}")
                    nc.sync.dma_start(out=wt[n][:], in_=wts[n][:, None])
                else:
                    wt[n] = ps.tile(shp, F32, name=f"wt_{n}")
                    nc.sync.dma_start(out=wt[n][:], in_=wts[n][:])


            for rp_i in range(reps):
              hf0 = dram.tile([GN, K0], BF16, addr_space="Shared",
                              name=f"hf0_{rp_i}")
              hfa = dram.tile([GN, H], BF16, addr_space="Shared",
                              name=f"hfa_{rp_i}")
              hfb = dram.tile([GN, H], BF16, addr_space="Shared",
                              name=f"hfb_{rp_i}")
              hown0_d = dram.tile([L, K0], BF16, name=f"hown0_{rp_i}")
              hown_d = dram.tile([L, H], BF16, name=f"hown_{rp_i}")
              st_ins = [dram.tile([H, 2], F32, name=f"st_in{i}_{rp_i}")
                        for i in range(4)]
              st_outs = [dram.tile([H, 2], F32, addr_space="Shared",
                                   name=f"st_out{i}_{rp_i}") for i in range(4)]
              # ---- h0: x || edge_agg (node-major)
              nc.gpsimd.dma_start(
                  out=h_own[:, :, 0:FN],
                  in_=xp_in[:].rearrange("(t p) f -> p t f", p=128))
              for t in range(NT):
                  d = int(Dp_t[t])
                  if d == 0:
                      nc.vector.memset(h_own[:, t, FN:K0], 0.0)
                      continue
                  ea_t = eap.tile([128, int(max(Dp_t)) * FE], BF16, tag="ea")
                  nc.sync.dma_start(
                      out=ea_t[:, :d * FE],
                      in_=ea_in[:, int(ea_off[t]) * FE:(int(ea_off[t]) + d) * FE])
                  sums = sm.tile([128, FE], F32, tag="easum")
                  nc.vector.tensor_reduce(
                      out=sums[:],
                      in_=ea_t[:, :d * FE].rearrange("p (d f) -> p f d", f=FE),
                      axis=mybir.AxisListType.X, op=mybir.AluOpType.add)
                  nc.vector.tensor_tensor(
                      out=h_own[:, t, FN:K0], in0=sums[:],
                      in1=icnt_sb[:, t:t + 1].to_broadcast([128, FE]),
                      op=mybir.AluOpType.mult)

              # AllGather h0 (virtual slots are zero by construction)
              nc.sync.dma_start(
                  out=hown0_d[:].rearrange("(t p) f -> p t f", p=128),
                  in_=h_own[:, :, 0:K0])
              nc.gpsimd.collective_compute(
                  "AllGather", mybir.AluOpType.bypass, replica_groups=rep,
                  ins=[hown0_d[:].opt()], outs=[hf0[:].opt()])

              # ---- GIN layers
              for l in range(3):
                  K = K0 if l == 0 else H
                  TAB = [hf0, hfa, hfb][l]
                  w1, b1 = wt[f"w1_{l}"], wt[f"b1_{l}"]
                  w2, b2 = wt[f"w2_{l}"], wt[f"b2_{l}"]
                  gam, bet = wt[f"g_{l}"], wt[f"be_{l}"]

                  # gather + segment reduce, per dest tile
                  for t in range(NT):
                      d = int(D_t[t])
                      if d == 0:
                          nc.vector.memset(acc[:, t, 0:K], 0.0)
                          continue
                      G = gp.tile([128, int(max(D_t)), H], BF16, tag="g")
                      for j in range(d):
                          o = int(col_off[t]) + j
                          nc.gpsimd.indirect_dma_start(
                              out=G[:, j, 0:K], out_offset=None,
                              in_=TAB[:],
                              in_offset=IndirectOffsetOnAxis(
                                  ap=idx_sb[:, o:o + 1], axis=0))
                      nc.vector.tensor_reduce(
                          out=acc[:, t, 0:K],
                          in_=G[:, 0:d, 0:K].rearrange("p d k -> p k d"),
                          axis=mybir.AxisListType.X, op=mybir.AluOpType.add)

                  # MLP feature-major, chunked over node columns; the
                  # z = h + agg add happens per tile so PE work for early
                  # chunks overlaps the remaining gathers
                  CH = 512
                  for c0 in range(0, L, CH):
                      cw = min(CH, L - c0)
                      nt0 = c0 // 128
                      zT = fm.tile([K, CH], F32, tag="zT")
                      for tt in range(nt0, nt0 + cw // 128):
                          nc.vector.tensor_add(
                              out=acc[:, tt, 0:K],
                              in0=acc[:, tt, 0:K],
                              in1=h_own[:, tt, 0:K])
                          tp = pp.tile([K, 128], F32, tag="tp", space="PSUM")
                          nc.tensor.transpose(
                              out=tp[:], in_=acc[:, tt, 0:K], identity=ident[:])
                          nc.vector.tensor_copy(
                              out=zT[:, (tt - nt0) * 128:(tt - nt0 + 1) * 128],
                              in_=tp[:])
                      mm1 = pp2.tile([H, CH], F32, tag="mm", space="PSUM")
                      nc.tensor.matmul(mm1[:, :cw], lhsT=w1[:], rhs=zT[:, :cw],
                                       start=True, stop=True)
                      uT = fm.tile([H, CH], F32, tag="uT")
                      nc.scalar.activation(uT[:, :cw], mm1[:, :cw],
                                           mybir.ActivationFunctionType.Relu,
                                           bias=b1[:])
                      mm2 = pp2.tile([H, CH], F32, tag="mm", space="PSUM")
                      nc.tensor.matmul(mm2[:, :cw], lhsT=w2[:], rhs=uT[:, :cw],
                                       start=True, stop=True)
                      nc.scalar.activation(z2T[:, c0:c0 + cw], mm2[:, :cw],
                                           mybir.ActivationFunctionType.Identity,
                                           bias=b2[:])

                  # zero pad-node columns, then BN stats
                  if L > cfg.real_local:
                      nc.vector.memset(z2T[:, cfg.real_local:L], 0.0)
                  sc1 = sm.tile([H, 1], F32, tag="sc1")
                  sc2 = sm.tile([H, 1], F32, tag="sc2")
                  nc.vector.memset(sc1[:], 0.0)
                  nc.vector.memset(sc2[:], 0.0)
                  for c0 in range(0, L, CH):
                      cw = min(CH, L - c0)
                      sdump = fm.tile([H, CH], F32, tag="sdump")
                      p1 = sm.tile([H, 1], F32, tag="p1")
                      nc.scalar.activation(sdump[:, :cw], z2T[:, c0:c0 + cw],
                                           mybir.ActivationFunctionType.Copy,
                                           accum_out=p1[:])
                      nc.vector.tensor_add(out=sc1[:], in0=sc1[:], in1=p1[:])
                      p2 = sm.tile([H, 1], F32, tag="p2")
                      nc.scalar.activation(sdump[:, :cw], z2T[:, c0:c0 + cw],
                                           mybir.ActivationFunctionType.Square,
                                           accum_out=p2[:])
                      nc.vector.tensor_add(out=sc2[:], in0=sc2[:], in1=p2[:])
                  stp = sm.tile([H, 2], F32, tag="stp")
                  nc.vector.tensor_copy(out=stp[:, 0:1], in_=sc1[:])
                  nc.vector.tensor_copy(out=stp[:, 1:2], in_=sc2[:])
                  nc.sync.dma_start(out=st_ins[l][:], in_=stp[:])
                  nc.gpsimd.collective_compute(
                      "AllReduce", mybir.AluOpType.add, replica_groups=rep,
                      ins=[st_ins[l][:].opt()], outs=[st_outs[l][:].opt()])
                  str_ = sm.tile([H, 2], F32, tag="str")
                  nc.sync.dma_start(out=str_[:], in_=st_outs[l][:])

                  # S = gamma * rsqrt(var + eps); B = beta - mu * S
                  mu = sm.tile([H, 1], F32, tag="mu")
                  nc.scalar.mul(mu[:], str_[:, 0:1], inv_n)
                  var = sm.tile([H, 1], F32, tag="var")
                  nc.vector.tensor_tensor(out=var[:], in0=mu[:], in1=mu[:],
                                          op=mybir.AluOpType.mult)
                  e2 = sm.tile([H, 1], F32, tag="e2")
                  nc.scalar.mul(e2[:], str_[:, 1:2], inv_n)
                  nc.vector.tensor_sub(out=var[:], in0=e2[:], in1=var[:])
                  epst = sm.tile([H, 1], F32, tag="epst")
                  nc.vector.memset(epst[:], BN_EPS)
                  sd = sm.tile([H, 1], F32, tag="sd")
                  nc.scalar.activation(sd[:], var[:],
                                       mybir.ActivationFunctionType.Sqrt,
                                       bias=epst[:])
                  rsd = sm.tile([H, 1], F32, tag="rsd")
                  nc.vector.reciprocal(rsd[:], sd[:])
                  Sg = sm.tile([H, 1], F32, tag="Sg")
                  nc.vector.tensor_tensor(out=Sg[:], in0=rsd[:], in1=gam[:],
                                          op=mybir.AluOpType.mult)
                  Bg = sm.tile([H, 1], F32, tag="Bg")
                  nc.vector.tensor_tensor(out=Bg[:], in0=mu[:], in1=Sg[:],
                                          op=mybir.AluOpType.mult)
                  nc.vector.tensor_sub(out=Bg[:], in0=bet[:], in1=Bg[:])

                  if l < 2:
                      # normalize + relu, transpose back node-major, AllGather
                      for c0 in range(0, L, CH):
                          cw = min(CH, L - c0)
                          hT = fm.tile([H, CH], F32, tag="hT")
                          nc.scalar.activation(
                              hT[:, :cw], z2T[:, c0:c0 + cw],
                              mybir.ActivationFunctionType.Relu,
                              bias=Bg[:].to_broadcast([H, 1]),
                              scale=Sg[:].to_broadcast([H, 1]))
                          if c0 + cw > cfg.real_local:
                              nc.vector.memset(
                                  hT[:, max(0, cfg.real_local - c0):cw], 0.0)
                          for tt in range(c0 // 128, (c0 + cw) // 128):
                              tb = pp.tile([128, H], F32, tag="tp", space="PSUM", name="tb")
                              nc.tensor.transpose(
                                  out=tb[:],
                                  in_=hT[:, (tt - c0 // 128) * 128:
                                         (tt - c0 // 128 + 1) * 128],
                                  identity=ident[0:H, 0:H])
                              nc.vector.tensor_copy(out=h_own[:, tt, :], in_=tb[:])
                      nc.sync.dma_start(
                          out=hown_d[:].rearrange("(t p) f -> p t f", p=128),
                          in_=h_own[:, :, :])
                      nc.gpsimd.collective_compute(
                          "AllGather", mybir.AluOpType.bypass,
                          replica_groups=rep,
                          ins=[hown_d[:].opt()],
                          outs=[(hfa if l == 0 else hfb)[:].opt()])
                  else:
                      # final: normalize+relu+pool (sum over real nodes)
                      pool = sm.tile([H, 1], F32, tag="pool")
                      nc.vector.memset(pool[:], 0.0)
                      for c0 in range(0, cfg.real_local, CH):
                          cw = min(CH, cfg.real_local - c0)
                          hT = fm.tile([H, CH], F32, tag="hT")
                          pc = sm.tile([H, 1], F32, tag="pc")
                          nc.scalar.activation(
                              hT[:, :cw], z2T[:, c0:c0 + cw],
                              mybir.ActivationFunctionType.Relu,
                              bias=Bg[:].to_broadcast([H, 1]),
                              scale=Sg[:].to_broadcast([H, 1]),
                              accum_out=pc[:])
                          nc.vector.tensor_add(out=pool[:], in0=pool[:],
                                               in1=pc[:])
                      stp2 = sm.tile([H, 2], F32, tag="stp")
                      nc.vector.memset(stp2[:], 0.0)
                      nc.vector.tensor_copy(out=stp2[:, 0:1], in_=pool[:])
                      nc.sync.dma_start(out=st_ins[3][:], in_=stp2[:])
                      nc.gpsimd.collective_compute(
                          "AllReduce", mybir.AluOpType.add, replica_groups=rep,
                          ins=[st_ins[3][:].opt()], outs=[st_outs[3][:].opt()])
                      strp = sm.tile([H, 2], F32, tag="str")
                      nc.sync.dma_start(out=strp[:], in_=st_outs[3][:])
                      pooled = sm.tile([H, 1], F32, tag="pooled")
                      nc.scalar.mul(pooled[:], strp[:, 0:1], inv_n)
                      # classifier
                      c1p = pp2.tile([H // 2, 1], F32, tag="mm", space="PSUM", name="c1p")
                      nc.tensor.matmul(c1p[:], lhsT=wt["cw1"][:], rhs=pooled[:],
                                       start=True, stop=True)
                      c1s = sm.tile([H // 2, 1], F32, tag="c1s")
                      nc.scalar.activation(c1s[:], c1p[:],
                                           mybir.ActivationFunctionType.Relu,
                                           bias=wt["cb1"][:])
                      c2p = pp2.tile([C, 1], F32, tag="mm", space="PSUM", name="c2p")
                      nc.tensor.matmul(c2p[:], lhsT=wt["cw2"][:], rhs=c1s[:],
                                       start=True, stop=True)
                      c2s = sm.tile([C, 1], F32, tag="c2s")
                      nc.scalar.activation(c2s[:], c2p[:],
                                           mybir.ActivationFunctionType.Identity,
                                           bias=wt["cb2"][:])
                      nc.sync.dma_start(out=out_ext[:], in_=c2s[:])

    nc.compile()
    return nc


def run(inputs, cfg, time_reps=False):
    gidx, ea_st, icnt, xp, meta = host_prep(inputs, cfg)
    nc = build_program(cfg, meta)
    in_maps = []
    for c in range(N_CORES):
        m = dict(gidx=gidx[c], ea_st=ea_st[c], icnt=icnt[c], xp=xp[c])
        for n in WNAMES:
            m[n] = np.asarray(inputs[n], np.float32)
        in_maps.append(m)
    res = run_bass_kernel_spmd(nc, in_maps, core_ids=list(range(N_CORES)))
    return res.results[0]["out"].astype(np.float32)


def kernel(**inputs) -> np.ndarray:
    return run(inputs, FULL)



# revision 14
# speedup vs baseline: 1.7021x; 1.7021x over previous
"""GIN (3-layer) over a 100K-node/1.6M-edge graph on 8 TRN2 NeuronCores.

Sharding: node g -> core g % 8 by global in-degree rank (so every core owns
12500 real nodes + 44 pad slots and per-tile gather depths are uniform
across cores). Edges are routed to the core owning the destination.

Gather strategy: the AllGathered h table [GN, 64] bf16 is viewed as
[GN/4, 256] -- four nodes packed per 512B row. dma_gather (InstDMAGatherAnt,
int16 group indices < 25088, 4 SWDGE queues) fetches one 512B row per edge
at the measured ~1.46ns/row descriptor rate; a host-precomputed {0,1} bf16
mask selects each edge's 64-wide sub-row during the DVE masked reduce that
forms the per-destination segment sums. This avoids both the ~1us/instr
fixed cost of per-column indirect DMAs (the old 7.4ms bottleneck) and the
2.5x padding inflation an int16 range-split of the 100352-row table would
cause.

Per GIN layer: gather+masked-reduce (node-major acc, bf16), z = h + agg,
MLP feature-major on the TensorEngine (PE transpose, two bf16 matmuls, ACT
bias+ReLU), BN stats via ACT accum_out + AllReduce, normalize+ReLU, PE
transpose back, AllGather the new table. Final global mean pool is an ACT
accum + AllReduce + replicated classifier.
"""

import numpy as np
import ml_dtypes

import concourse.bass as bass
import concourse.bacc as bacc
import concourse.tile as tile
import concourse.mybir as mybir
from concourse.bass_utils import run_bass_kernel_spmd
from concourse.masks import make_identity

F32 = mybir.dt.float32
BF16 = mybir.dt.bfloat16
I32 = mybir.dt.int32
I16 = mybir.dt.int16

N_CORES = 8
FN, FE, H, C = 32, 16, 64, 2
BN_EPS = 1e-5
PACK = 4           # nodes per 512B table row
GCOLS = 8          # gather columns per dma_gather instr (<= 1024 idxs)


class Cfg:
    def __init__(self, n_real, local):
        assert local % 128 == 0
        self.n_real = n_real
        self.local = local
        self.real_local = n_real // N_CORES
        self.nt = local // 128
        self.gn = N_CORES * local


FULL = Cfg(100_000, 12_544)


# --------------------------------------------------------------------------
# Host preprocessing
# --------------------------------------------------------------------------

def host_prep(inputs, cfg):
    E = inputs["edge_index"].shape[1]
    row = np.asarray(inputs["edge_index"][0])
    col = np.asarray(inputs["edge_index"][1])
    L, NT = cfg.local, cfg.nt

    degg = np.bincount(col, minlength=cfg.n_real)
    grank = np.argsort(-degg, kind="stable")
    rank_of = np.empty(cfg.n_real, np.int64)
    rank_of[grank] = np.arange(cfg.n_real)
    core_of = (rank_of % N_CORES).astype(np.int32)
    slot_of_global = (rank_of // N_CORES).astype(np.int32)
    degs = np.zeros((N_CORES, L), np.int64)
    degs[core_of, slot_of_global] = degg

    # per-tile gather depth, shared across cores (SPMD program is shared)
    D_t = degs.reshape(N_CORES, NT, 128).max(axis=(0, 2)).astype(np.int64)
    S = int(D_t.sum())
    col_off = np.zeros(NT, np.int64)
    np.cumsum(D_t[:-1], out=col_off[1:])

    tab_row = (core_of.astype(np.int64) * L
               + slot_of_global).astype(np.int64)

    # per-core gather index (table row) per (dest-partition, column), with
    # pad entries marked -1
    gidx_full = np.full((N_CORES, 128, max(S, 1)), -1, np.int64)
    for c in range(N_CORES):
        m = core_of[col] == c
        e = np.where(m)[0]
        slot_e = slot_of_global[col[e]]
        o = np.argsort(slot_e, kind="stable")
        e, slot_e = e[o], slot_e[o]
        start = np.zeros(L + 1, np.int64)
        np.cumsum(np.bincount(slot_e, minlength=L), out=start[1:])
        j = np.arange(len(e)) - start[slot_e]
        t_e = slot_e // 128
        p_e = slot_e % 128
        gidx_full[c, p_e, col_off[t_e] + j] = tab_row[row[e]]

    # chunk schedule: per tile, groups of <= GCOLS columns
    chunks = []      # (tile, j0, cols, idx_off_i16cols, mask_off)
    idx_cols = 0
    for t in range(NT):
        d = int(D_t[t])
        j0 = 0
        while j0 < d:
            cseg = min(GCOLS, d - j0)
            chunks.append((t, j0, cseg, idx_cols, (int(col_off[t]) + j0) * PACK))
            idx_cols += cseg * 128 // 16
            j0 += cseg

    # int16 wrapped group-index stream + bf16 sub-row masks
    gidx16 = np.zeros((N_CORES, 128, max(idx_cols, 1)), np.int16)
    gmask = np.zeros((N_CORES, 128, max(S, 1) * PACK), ml_dtypes.bfloat16)
    for c in range(N_CORES):
        rows = gidx_full[c]                      # [128, S]
        grp = np.where(rows >= 0, rows // PACK, 0).astype(np.int16)
        sub = (rows % PACK).astype(np.int64)
        valid = rows >= 0
        # masks, global-column-major: mask[p, (col*PACK + m)]
        mk = np.zeros((128, max(S, 1), PACK), np.float32)
        pp, cc = np.nonzero(valid)
        mk[pp, cc, sub[pp, cc]] = 1.0
        gmask[c] = mk.reshape(128, -1).astype(ml_dtypes.bfloat16)
        for (t, j0, cseg, io, mo) in chunks:
            blk = grp[:, int(col_off[t]) + j0:int(col_off[t]) + j0 + cseg]
            # position i = j_local*128 + p -> wrapped [i%16, i//16], x8
            flat = blk.T.reshape(-1)             # i = j*128 + p
            w = flat.reshape(cseg * 8, 16).T     # [16, cseg*8]
            gidx16[c, :, io:io + cseg * 8] = np.tile(w, (8, 1))

    # edge_attr scatter-mean (dest = row): per-slot layout
    rdegs = []
    slotr = []
    for c in range(N_CORES):
        m = core_of[row] == c
        e = np.where(m)[0]
        slot_e = slot_of_global[row[e]]
        rd = np.bincount(slot_e, minlength=L)
        rdegs.append(rd)
        slotr.append((e, slot_e))
    rdegs = np.stack(rdegs)
    Dp_t = rdegs.reshape(N_CORES, NT, 128).max(axis=(0, 2)).astype(np.int64)
    SEA = int(Dp_t.sum())
    ea_off = np.zeros(NT, np.int64)
    np.cumsum(Dp_t[:-1], out=ea_off[1:])
    Dpmax = int(Dp_t.max()) if SEA else 0

    ea = np.asarray(inputs["edge_attr"]).astype(ml_dtypes.bfloat16)
    ea_st = np.zeros((N_CORES, 128, max(SEA, 1) * FE), ml_dtypes.bfloat16)
    icnt = np.zeros((N_CORES, 128, NT), np.float32)
    for c in range(N_CORES):
        e, slot_e = slotr[c]
        o = np.argsort(slot_e, kind="stable")
        e, slot_e = e[o], slot_e[o]
        start = np.zeros(L + 1, np.int64)
        np.cumsum(np.bincount(slot_e, minlength=L), out=start[1:])
        j = np.arange(len(e)) - start[slot_e]
        A = np.zeros((L, Dpmax, FE), ml_dtypes.bfloat16)
        A[slot_e, j, :] = ea[e]
        for t in range(NT):
            d = int(Dp_t[t])
            if d:
                blk = A[t * 128:(t + 1) * 128, :d, :].reshape(128, d * FE)
                ea_st[c, :, int(ea_off[t]) * FE:(int(ea_off[t]) + d) * FE] = blk
        icnt[c, :, :] = (1.0 / np.maximum(
            rdegs[c].reshape(NT, 128), 1.0)).T.astype(np.float32)

    x = np.asarray(inputs["x"], np.float32)
    xp = np.zeros((N_CORES, L, FN), np.float32)
    xp[core_of, slot_of_global] = x

    meta = dict(D_t=D_t, S=S, Dp_t=Dp_t, SEA=SEA, col_off=col_off,
                ea_off=ea_off, chunks=chunks, idx_cols=idx_cols)
    return gidx16, gmask, ea_st, icnt, xp, meta


# --------------------------------------------------------------------------
# Device program
# --------------------------------------------------------------------------

WNAMES = []
for _l in range(3):
    WNAMES += [f"w1_{_l}", f"b1_{_l}", f"w2_{_l}", f"b2_{_l}", f"g_{_l}", f"be_{_l}"]
WNAMES += ["cw1", "cb1", "cw2", "cb2"]


def _chunk_tiles(D, nt, max_cols):
    off = np.zeros(nt + 1, np.int64)
    np.cumsum(np.asarray(D[:nt], np.int64), out=off[1:])
    out = []
    t = 0
    while t < nt:
        t0 = t
        cols = 0
        while t < nt and (cols + int(D[t]) <= max_cols or t == t0):
            cols += int(D[t])
            t += 1
        out.append((t0, t, int(off[t0]), cols))
    return out


def build_program(cfg, meta, reps=1):
    L, NT, GN = cfg.local, cfg.nt, cfg.gn
    D_t, S = meta["D_t"], meta["S"]
    Dp_t = meta["Dp_t"]
    SEA = meta["SEA"]
    col_off, ea_off = meta["col_off"], meta["ea_off"]
    chunks, idx_cols = meta["chunks"], meta["idx_cols"]
    inv_n = 1.0 / cfg.n_real
    K0 = FN + FE

    nc = bacc.Bacc("TRN2", target_bir_lowering=False, debug=False,
                   num_devices=N_CORES, num_swdge_queues=4)

    gidx_in = nc.dram_tensor("gidx16", [128, max(idx_cols, 1)], I16,
                             kind="ExternalInput")
    gmask_in = nc.dram_tensor("gmask", [128, max(S, 1) * PACK], BF16,
                              kind="ExternalInput")
    ea_in = nc.dram_tensor("ea_st", [128, max(SEA, 1) * FE], BF16,
                           kind="ExternalInput")
    icnt_in = nc.dram_tensor("icnt", [128, NT], F32, kind="ExternalInput")
    xp_in = nc.dram_tensor("xp", [L, FN], F32, kind="ExternalInput")
    wts = {}
    wshapes = {}
    for l in range(3):
        kin = K0 if l == 0 else H
        wshapes[f"w1_{l}"] = [kin, H]
        wshapes[f"b1_{l}"] = [H]
        wshapes[f"w2_{l}"] = [H, H]
        wshapes[f"b2_{l}"] = [H]
        wshapes[f"g_{l}"] = [H]
        wshapes[f"be_{l}"] = [H]
    wshapes.update(cw1=[H, H // 2], cb1=[H // 2], cw2=[H // 2, C], cb2=[C])
    for n in WNAMES:
        wts[n] = nc.dram_tensor(n, wshapes[n], F32, kind="ExternalInput")
    out_ext = nc.dram_tensor("out", [1, C], F32, kind="ExternalOutput")

    rep = [list(range(N_CORES))]

    with tile.TileContext(nc) as tc:
        with (
            nc.allow_low_precision(
                reason="bf16 segment sums stay well inside the 2e-2 "
                       "relative-error budget (max |sum| ~ 40, bf16 eps "
                       "~ 0.4%)"),
            tc.tile_pool(name="dram", bufs=1, space="DRAM") as dram,
            tc.tile_pool(name="persist", bufs=1) as ps,
            tc.tile_pool(name="gpool", bufs=8) as gp,
            tc.tile_pool(name="mpool", bufs=1) as mp,
            tc.tile_pool(name="ea", bufs=2) as eap,
            tc.tile_pool(name="fm", bufs=3) as fm,
            tc.tile_pool(name="small", bufs=2) as sm,
            tc.tile_pool(name="psum", bufs=2, space="PSUM") as pp,
            tc.tile_pool(name="psum2", bufs=2, space="PSUM") as pp2,
        ):
            # ---- persistent SBUF
            idx_sb = ps.tile([128, max(idx_cols, 1)], I16)
            nc.sync.dma_start(out=idx_sb[:], in_=gidx_in[:])
            mask_sb = ps.tile([128, max(S, 1) * PACK], BF16)
            nc.sync.dma_start(out=mask_sb[:], in_=gmask_in[:])
            h_own = ps.tile([128, NT, H], BF16)      # node-major h (64 wide)
            acc = ps.tile([128, NT, H], BF16)        # agg / z node-major
            z2T = ps.tile([H, L], BF16)              # feature-major MLP out
            icnt_sb = ps.tile([128, NT], F32)
            nc.sync.dma_start(out=icnt_sb[:], in_=icnt_in[:])
            ident = ps.tile([128, 128], BF16)
            make_identity(nc, ident[:])
            wt = {}
            for n in WNAMES:
                shp = wshapes[n]
                if len(shp) == 1:
                    wt[n] = ps.tile([shp[0], 1], F32, name=f"wt_{n}")
                    nc.sync.dma_start(out=wt[n][:], in_=wts[n][:, None])
                elif n in ("cw1", "cw2"):
                    wt[n] = ps.tile(shp, F32, name=f"wt_{n}")
                    nc.sync.dma_start(out=wt[n][:], in_=wts[n][:])
                else:
                    wf = sm.tile(shp, F32, tag="wload", name=f"wl_{n}")
                    nc.sync.dma_start(out=wf[:], in_=wts[n][:])
                    wt[n] = ps.tile(shp, BF16, name=f"wt_{n}")
                    nc.vector.tensor_copy(out=wt[n][:], in_=wf[:])

            for rp_i in range(reps):
                hf = [dram.tile([GN, H], BF16, addr_space="Shared",
                                name=f"hf{i}_{rp_i}") for i in range(3)]
                hown_d = dram.tile([L, H], BF16, name=f"hown_{rp_i}")
                st_ins = [dram.tile([H, 2], F32, name=f"st_in{i}_{rp_i}")
                          for i in range(4)]
                st_outs = [dram.tile([H, 2], F32, addr_space="Shared",
                                     name=f"st_out{i}_{rp_i}")
                           for i in range(4)]

                # ---- h0: x || edge_agg (node-major); cols 48:64 stay zero
                nc.vector.memset(h_own[:, :, K0:H], 0.0)
                nc.gpsimd.dma_start(
                    out=h_own[:, :, 0:FN],
                    in_=xp_in[:].rearrange("(t p) f -> p t f", p=128))
                ea_chunks = _chunk_tiles(Dp_t, NT, 256)
                ea_maxc = max(c[3] for c in ea_chunks)
                for (t0, t1, c0, cols) in ea_chunks:
                    if cols == 0:
                        for t in range(t0, t1):
                            nc.vector.memset(h_own[:, t, FN:K0], 0.0)
                        continue
                    ea_t = eap.tile([128, ea_maxc * FE], BF16, tag="ea")
                    nc.sync.dma_start(
                        out=ea_t[:, :cols * FE],
                        in_=ea_in[:, c0 * FE:(c0 + cols) * FE])
                    for t in range(t0, t1):
                        d = int(Dp_t[t])
                        if d == 0:
                            nc.vector.memset(h_own[:, t, FN:K0], 0.0)
                            continue
                        o = int(ea_off[t]) - c0
                        sums = sm.tile([128, FE], F32, tag="easum")
                        nc.vector.tensor_reduce(
                            out=sums[:],
                            in_=ea_t[:, o * FE:(o + d) * FE].rearrange(
                                "p (d f) -> p f d", f=FE),
                            axis=mybir.AxisListType.X, op=mybir.AluOpType.add)
                        nc.vector.tensor_tensor(
                            out=h_own[:, t, FN:K0], in0=sums[:],
                            in1=icnt_sb[:, t:t + 1].to_broadcast([128, FE]),
                            op=mybir.AluOpType.mult)

                # AllGather h0 (virtual slots zero by construction)
                nc.sync.dma_start(
                    out=hown_d[:].rearrange("(t p) f -> p t f", p=128),
                    in_=h_own[:, :, :])
                nc.gpsimd.collective_compute(
                    "AllGather", mybir.AluOpType.bypass, replica_groups=rep,
                    ins=[hown_d[:].opt()], outs=[hf[0][:].opt()])

                # ---- GIN layers
                for l in range(3):
                    TAB = hf[l][:].rearrange("(g four) k -> g (four k)",
                                             four=PACK)
                    w1, b1 = wt[f"w1_{l}"], wt[f"b1_{l}"]
                    w2, b2 = wt[f"w2_{l}"], wt[f"b2_{l}"]
                    gam, bet = wt[f"g_{l}"], wt[f"be_{l}"]
                    K = K0 if l == 0 else H

                    # gather (4-packed 512B rows) + masked segment reduce
                    for ci, (t, j0, cseg, io, mo) in enumerate(chunks):
                        G = gp.tile([128, GCOLS, PACK * H], BF16, tag="g")
                        nc.gpsimd.dma_gather(
                            G[:, 0:cseg, :], TAB,
                            idx_sb[:, io:io + cseg * 8],
                            cseg * 128, cseg * 128, PACK * H,
                            queue_num=ci % 4)
                        msk = mp.tile([128, GCOLS * PACK * H], BF16,
                                      tag="msk")
                        nc.vector.tensor_tensor(
                            out=msk[:, 0:cseg * PACK * H].rearrange(
                                "p (d k) -> p d k", k=H),
                            in0=G[:, 0:cseg, :].rearrange(
                                "p c (m k) -> p (c m) k", k=H),
                            in1=mask_sb[:, mo:mo + cseg * PACK, None]
                                .to_broadcast([128, cseg * PACK, H]),
                            op=mybir.AluOpType.mult)
                        if j0 == 0:
                            nc.vector.tensor_reduce(
                                out=acc[:, t, :],
                                in_=msk[:, 0:cseg * PACK * H].rearrange(
                                    "p (d k) -> p k d", k=H),
                                axis=mybir.AxisListType.X,
                                op=mybir.AluOpType.add)
                        else:
                            part = sm.tile([128, H], BF16, tag="part")
                            nc.vector.tensor_reduce(
                                out=part[:],
                                in_=msk[:, 0:cseg * PACK * H].rearrange(
                                    "p (d k) -> p k d", k=H),
                                axis=mybir.AxisListType.X,
                                op=mybir.AluOpType.add)
                            nc.vector.tensor_add(
                                out=acc[:, t, :], in0=acc[:, t, :],
                                in1=part[:])

                    # MLP feature-major, chunked over node columns
                    CH = 512
                    for c0 in range(0, L, CH):
                        cw = min(CH, L - c0)
                        nt0 = c0 // 128
                        zT = fm.tile([H, CH], BF16, tag="zT")
                        for tt in range(nt0, nt0 + cw // 128):
                            nc.vector.tensor_add(
                                out=acc[:, tt, :],
                                in0=acc[:, tt, :],
                                in1=h_own[:, tt, :])
                            tp = pp.tile([H, 128], BF16, tag="tp",
                                         space="PSUM")
                            nc.tensor.transpose(
                                out=tp[:], in_=acc[:, tt, :],
                                identity=ident[:])
                            nc.vector.tensor_copy(
                                out=zT[:, (tt - nt0) * 128:
                                       (tt - nt0 + 1) * 128],
                                in_=tp[:])
                        mm1 = pp2.tile([H, CH], F32, tag="mm", space="PSUM")
                        nc.tensor.matmul(mm1[:, :cw], lhsT=w1[0:K, :],
                                         rhs=zT[0:K, :cw],
                                         start=True, stop=True)
                        uT = fm.tile([H, CH], BF16, tag="uT")
                        nc.scalar.activation(uT[:, :cw], mm1[:, :cw],
                                             mybir.ActivationFunctionType.Relu,
                                             bias=b1[:])
                        mm2 = pp2.tile([H, CH], F32, tag="mm", space="PSUM")
                        nc.tensor.matmul(mm2[:, :cw], lhsT=w2[:],
                                         rhs=uT[:, :cw],
                                         start=True, stop=True)
                        nc.scalar.activation(z2T[:, c0:c0 + cw], mm2[:, :cw],
                                             mybir.ActivationFunctionType.Identity,
                                             bias=b2[:])

                    # zero pad-node columns, then BN stats
                    if L > cfg.real_local:
                        nc.vector.memset(z2T[:, cfg.real_local:L], 0.0)
                    sc1 = sm.tile([H, 1], F32, tag="sc1")
                    sc2 = sm.tile([H, 1], F32, tag="sc2")
                    nc.vector.memset(sc1[:], 0.0)
                    nc.vector.memset(sc2[:], 0.0)
                    for c0 in range(0, L, CH):
                        cw = min(CH, L - c0)
                        sdump = fm.tile([H, CH], BF16, tag="sdump")
                        p1 = sm.tile([H, 1], F32, tag="p1")
                        nc.scalar.activation(sdump[:, :cw], z2T[:, c0:c0 + cw],
                                             mybir.ActivationFunctionType.Copy,
                                             accum_out=p1[:])
                        nc.vector.tensor_add(out=sc1[:], in0=sc1[:], in1=p1[:])
                        p2 = sm.tile([H, 1], F32, tag="p2")
                        nc.scalar.activation(sdump[:, :cw], z2T[:, c0:c0 + cw],
                                             mybir.ActivationFunctionType.Square,
                                             accum_out=p2[:])
                        nc.vector.tensor_add(out=sc2[:], in0=sc2[:], in1=p2[:])
                    stp = sm.tile([H, 2], F32, tag="stp")
                    nc.vector.tensor_copy(out=stp[:, 0:1], in_=sc1[:])
                    nc.vector.tensor_copy(out=stp[:, 1:2], in_=sc2[:])
                    nc.sync.dma_start(out=st_ins[l][:], in_=stp[:])
                    nc.gpsimd.collective_compute(
                        "AllReduce", mybir.AluOpType.add, replica_groups=rep,
                        ins=[st_ins[l][:].opt()], outs=[st_outs[l][:].opt()])
                    str_ = sm.tile([H, 2], F32, tag="str")
                    nc.sync.dma_start(out=str_[:], in_=st_outs[l][:])

                    # Sg = gamma * rsqrt(var + eps); Bg = beta - mu * Sg
                    mu = sm.tile([H, 1], F32, tag="mu")
                    nc.scalar.mul(mu[:], str_[:, 0:1], inv_n)
                    var = sm.tile([H, 1], F32, tag="var")
                    nc.vector.tensor_tensor(out=var[:], in0=mu[:], in1=mu[:],
                                            op=mybir.AluOpType.mult)
                    e2 = sm.tile([H, 1], F32, tag="e2")
                    nc.scalar.mul(e2[:], str_[:, 1:2], inv_n)
                    nc.vector.tensor_sub(out=var[:], in0=e2[:], in1=var[:])
                    epst = sm.tile([H, 1], F32, tag="epst")
                    nc.vector.memset(epst[:], BN_EPS)
                    sd = sm.tile([H, 1], F32, tag="sd")
                    nc.scalar.activation(sd[:], var[:],
                                         mybir.ActivationFunctionType.Sqrt,
                                         bias=epst[:])
                    rsd = sm.tile([H, 1], F32, tag="rsd")
                    nc.vector.reciprocal(rsd[:], sd[:])
                    Sg = sm.tile([H, 1], F32, tag="Sg")
                    nc.vector.tensor_tensor(out=Sg[:], in0=rsd[:], in1=gam[:],
                                            op=mybir.AluOpType.mult)
                    Bg = sm.tile([H, 1], F32, tag="Bg")
                    nc.vector.tensor_tensor(out=Bg[:], in0=mu[:], in1=Sg[:],
                                            op=mybir.AluOpType.mult)
                    nc.vector.tensor_sub(out=Bg[:], in0=bet[:], in1=Bg[:])

                    if l < 2:
                        # normalize + relu, transpose back, AllGather
                        for c0 in range(0, L, CH):
                            cw = min(CH, L - c0)
                            hT = fm.tile([H, CH], BF16, tag="hT")
                            nc.scalar.activation(
                                hT[:, :cw], z2T[:, c0:c0 + cw],
                                mybir.ActivationFunctionType.Relu,
                                bias=Bg[:].to_broadcast([H, 1]),
                                scale=Sg[:].to_broadcast([H, 1]))
                            if c0 + cw > cfg.real_local:
                                nc.vector.memset(
                                    hT[:, max(0, cfg.real_local - c0):cw], 0.0)
                            for tt in range(c0 // 128, (c0 + cw) // 128):
                                tb = pp.tile([128, H], BF16, tag="tp",
                                             space="PSUM", name="tb")
                                nc.tensor.transpose(
                                    out=tb[:],
                                    in_=hT[:, (tt - c0 // 128) * 128:
                                           (tt - c0 // 128 + 1) * 128],
                                    identity=ident[0:H, 0:H])
                                nc.vector.tensor_copy(out=h_own[:, tt, :],
                                                      in_=tb[:])
                        nc.sync.dma_start(
                            out=hown_d[:].rearrange("(t p) f -> p t f", p=128),
                            in_=h_own[:, :, :])
                        nc.gpsimd.collective_compute(
                            "AllGather", mybir.AluOpType.bypass,
                            replica_groups=rep,
                            ins=[hown_d[:].opt()],
                            outs=[hf[l + 1][:].opt()])
                    else:
                        # final: normalize+relu+pool (sum over real nodes)
                        pool = sm.tile([H, 1], F32, tag="pool")
                        nc.vector.memset(pool[:], 0.0)
                        for c0 in range(0, cfg.real_local, CH):
                            cw = min(CH, cfg.real_local - c0)
                            hT = fm.tile([H, CH], BF16, tag="hT")
                            pc = sm.tile([H, 1], F32, tag="pc")
                            nc.scalar.activation(
                                hT[:, :cw], z2T[:, c0:c0 + cw],
                                mybir.ActivationFunctionType.Relu,
                                bias=Bg[:].to_broadcast([H, 1]),
                                scale=Sg[:].to_broadcast([H, 1]),
                                accum_out=pc[:])
                            nc.vector.tensor_add(out=pool[:], in0=pool[:],
                                                 in1=pc[:])
                        stp2 = sm.tile([H, 2], F32, tag="stp")
                        nc.vector.memset(stp2[:], 0.0)
                        nc.vector.tensor_copy(out=stp2[:, 0:1], in_=pool[:])
                        nc.sync.dma_start(out=st_ins[3][:], in_=stp2[:])
                        nc.gpsimd.collective_compute(
                            "AllReduce", mybir.AluOpType.add,
                            replica_groups=rep,
                            ins=[st_ins[3][:].opt()],
                            outs=[st_outs[3][:].opt()])
                        strp = sm.tile([H, 2], F32, tag="str")
                        nc.sync.dma_start(out=strp[:], in_=st_outs[3][:])
                        pooled = sm.tile([H, 1], F32, tag="pooled")
                        nc.scalar.mul(pooled[:], strp[:, 0:1], inv_n)
                        # classifier (f32, tiny)
                        c1p = pp2.tile([H // 2, 1], F32, tag="mm",
                                       space="PSUM", name="c1p")
                        nc.tensor.matmul(c1p[:], lhsT=wt["cw1"][:],
                                         rhs=pooled[:], start=True, stop=True)
                        c1s = sm.tile([H // 2, 1], F32, tag="c1s")
                        nc.scalar.activation(c1s[:], c1p[:],
                                             mybir.ActivationFunctionType.Relu,
                                             bias=wt["cb1"][:])
                        c2p = pp2.tile([C, 1], F32, tag="mm", space="PSUM",
                                       name="c2p")
                        nc.tensor.matmul(c2p[:], lhsT=wt["cw2"][:], rhs=c1s[:],
                                         start=True, stop=True)
                        c2s = sm.tile([C, 1], F32, tag="c2s")
                        nc.scalar.activation(c2s[:], c2p[:],
                                             mybir.ActivationFunctionType.Identity,
                                             bias=wt["cb2"][:])
                        nc.sync.dma_start(out=out_ext[:], in_=c2s[:])

    nc.compile()
    return nc


def run(inputs, cfg):
    gidx16, gmask, ea_st, icnt, xp, meta = host_prep(inputs, cfg)
    nc = build_program(cfg, meta)
    in_maps = []
    for c in range(N_CORES):
        m = dict(gidx16=gidx16[c], gmask=gmask[c], ea_st=ea_st[c],
                 icnt=icnt[c], xp=xp[c])
        for n in WNAMES:
            m[n] = np.asarray(inputs[n], np.float32)
        in_maps.append(m)
    res = run_bass_kernel_spmd(nc, in_maps, core_ids=list(range(N_CORES)))
    return res.results[0]["out"].astype(np.float32)


def kernel(**inputs) -> np.ndarray:
    return run(inputs, FULL)
